# revision 27
# baseline (speedup 1.0000x reference)
"""DGCN (diffusion graph conv) Trainium2 Bass kernel.

Reference computation (per batch b, time t):
    h   = relu(st_emb @ W1 + b1)              # [T,1,32]
    lam = 1 + relu(h @ W2 + b2)               # [T,1,1]
    c1  = 2 - 2/lam ;  c2 = 2/lam             # scalars per t
    out[b,t] = c1[t] * (x[b,t] @ W0) + c2[t] * ((adj @ x[b,t]) @ W1g) + bias
where weights = [W0; W1g] with shape [2, 64, 64].

Strategy: data-parallel over batch B=8 across the 8 NeuronCores (adj and
weights replicated).  Per core both output weight applications are
pre-folded once (associativity: (adj@x)@(c2 W1g) == adj@(x@(c2 W1g))):

  prologue (runs once; z' and u persist in SBUF):
    z'[j, t*64+oc] = sum_ic x[t,j,ic] * W1g[ic,oc] * c2[t]   (node-major)
    u[t*64+oc, n]  = c1[t] * sum_ic x[t,n,ic] * W0[ic,oc] + bias[oc]
  per iteration (4 column chunks of 512 nodes, k-major over 16 k-tiles):
    yt[m] = sum_k z'node[k-tile, m-slice].T @ adjT[k-tile, chunk]  (96 mm)
    out[m] = yt[m] + u[m, chunk]        (DVE drain straight from PSUM)

Per-iteration PE work is therefore the bare adjacency contraction:
196,608 f32r cycles (1 cycle/row).  PSUM runs as a single 8-bank ring so
a draining bank never blocks the next chunk's accumulation; drains and
output DMA ride the DVE/scalar paths which are otherwise idle.
"""
import numpy as np

import concourse.bass as bass
import concourse.tile as tile
from concourse import bacc, mybir
from concourse.bass_utils import run_bass_kernel_spmd

# Problem shapes (hardcoded per the harness contract).
B, T, N, C = 8, 12, 2048, 64
TC = T * C                     # 768
P = 128                        # partitions
KT = N // P                    # 16 k tiles
NCHUNK = 512                   # node columns per chunk
CHUNKS = N // NCHUNK           # 4
MT = TC // P                   # 6 tc (pair-of-timestep) tiles
NPAIR = T // 2                 # 6

F32 = mybir.dt.float32
F32R = mybir.dt.float32r


def build_kernel(repeat=1):
    nc = bacc.Bacc(name="dgcn")

    # ---- per-core external inputs -------------------------------------
    adjt = nc.dram_tensor("adjt", [N, N], F32R, kind="ExternalInput")
    # channel-major x pair-slabs: xcm[a*64+c, m, n] = x[2m+a, n, c]
    xcm = nc.dram_tensor("xcm", [P, MT, N], F32R, kind="ExternalInput")
    # all small constants packed into one blob so startup is a single HWDGE
    # DMA instead of a dozen ~1us SWDGE descriptor dispatches.
    # column layout (see prep_in_maps): sT[0:64,0:12] w1[0:64,12:44]
    # b1[0:32,44] w2[0:32,45] b2[0:1,46] bias[0:128,47] mask_up[0:1,48:176]
    # mask_lo[0:1,176:304] wd0[:,304:432] wd1[:,432:560] ident[:,560:688]
    cblob = nc.dram_tensor("cblob", [P, 688], F32, kind="ExternalInput")
    out = nc.dram_tensor("out", [TC, N], F32, kind="ExternalOutput")

    out_ap = out.ap().rearrange("(m p) n -> p m n", p=P)

    with tile.TileContext(nc) as tc:
        with (
            tc.tile_pool(name="const", bufs=1) as const,
            tc.tile_pool(name="xcs", bufs=2) as xcs_pool,
            tc.tile_pool(name="zc", bufs=3) as zc_pool,
            tc.tile_pool(name="adj", bufs=5) as adj_pool,
            tc.tile_pool(name="outs", bufs=6) as outs_pool,
            tc.tile_pool(name="ps", bufs=8, space="PSUM") as ps_pool,
        ):
            # ============ constants / lambda MLP =============
            cb_sb = const.tile([P, 688], F32)
            sT_sb = cb_sb[:64, 0:T]
            w1_sb = cb_sb[:64, 12:44]
            b1_sb = cb_sb[:32, 44:45]
            w2_sb = cb_sb[:32, 45:46]
            b2_sb = cb_sb[:1, 46:47]
            bias_sb = cb_sb[:, 47:48]
            mask_up = cb_sb[:1, 48:176]
            mask_lo = cb_sb[:1, 176:304]
            wd0_sb = cb_sb[:, 304:432]
            wd1_sb = cb_sb[:, 432:560]
            ident_sb = cb_sb[:, 560:688]
            # resident prologue products
            znode_sb = const.tile([P, KT, TC], F32R)   # node-major z'
            u_sb = const.tile([P, MT, N], F32)         # c1*x@W0 + bias

            def load_consts():
                nc.sync.dma_start(cb_sb[:], cblob.ap())

            def ps_tile(name, dtype=F32):
                return ps_pool.tile([P, NCHUNK], dtype, tag="ps", name=name)

            # scaled block-diagonal stationaries (filled by emit_mlp)
            wx_sb = const.tile([P, NPAIR, P], F32R)   # c1-scaled W0 pair-diag
            wz_sb = const.tile([P, NPAIR, P], F32R)   # c2-scaled W1g pair-diag

            def emit_mlp():
                """Lambda MLP + paired scaled weight stationaries."""
                # h.T = relu(W1.T @ sT + b1)   [32, T]
                h_ps = ps_tile("h_ps")
                nc.tensor.matmul(h_ps[:32, :T], w1_sb[:], sT_sb[:], start=True, stop=True)
                hr_sb = const.tile([32, T], F32)
                nc.scalar.activation(out=hr_sb[:], in_=h_ps[:32, :T],
                                     func=mybir.ActivationFunctionType.Relu,
                                     bias=b1_sb[:], scale=1.0)
                # lam = 1 + relu(W2.T @ hr + b2)   [1, T]
                lam_ps = ps_tile("lam_ps")
                nc.tensor.matmul(lam_ps[:1, :T], w2_sb[:], hr_sb[:], start=True, stop=True)
                lam_sb = const.tile([1, T], F32)
                nc.scalar.activation(out=lam_sb[:], in_=lam_ps[:1, :T],
                                     func=mybir.ActivationFunctionType.Relu,
                                     bias=b2_sb[:], scale=1.0)
                lam1_sb = const.tile([1, T], F32)
                nc.vector.tensor_scalar_add(lam1_sb[:], lam_sb[:], 1.0)
                inv_sb = const.tile([1, T], F32)
                nc.vector.reciprocal(out=inv_sb[:], in_=lam1_sb[:])
                c2_sb = const.tile([1, T], F32)
                nc.vector.tensor_scalar_mul(c2_sb[:], inv_sb[:], 2.0)
                c1_sb = const.tile([1, T], F32)
                nc.vector.tensor_scalar(c1_sb[:], inv_sb[:], -2.0, 2.0,
                                        mybir.AluOpType.mult, mybir.AluOpType.add)

                # paired per-partition coefficient columns:
                # cp[:, m] = [c1[2m]]*64 + [c1[2m+1]]*64, same for c2.
                cp_ps = ps_tile("cp_ps")
                c1_pairs = c1_sb.rearrange("p (a two) -> p two a", two=2)
                c2_pairs = c2_sb.rearrange("p (a two) -> p two a", two=2)
                nc.tensor.matmul(cp_ps[:, :NPAIR], mask_up[:], c1_pairs[:, 0, :],
                                 start=True, stop=False)
                nc.tensor.matmul(cp_ps[:, :NPAIR], mask_lo[:], c1_pairs[:, 1, :],
                                 start=False, stop=False)
                nc.tensor.matmul(cp_ps[:, NPAIR:2 * NPAIR], mask_up[:], c2_pairs[:, 0, :],
                                 start=False, stop=False)
                nc.tensor.matmul(cp_ps[:, NPAIR:2 * NPAIR], mask_lo[:], c2_pairs[:, 1, :],
                                 start=False, stop=True)
                cp_sb = const.tile([P, 2 * NPAIR], F32)
                nc.vector.tensor_copy(out=cp_sb[:], in_=cp_ps[:, :2 * NPAIR])

                for m in range(NPAIR):
                    nc.vector.tensor_scalar_mul(wx_sb[:, m, :], wd0_sb[:], cp_sb[:, m:m + 1])
                    nc.vector.tensor_scalar_mul(wz_sb[:, m, :], wd1_sb[:],
                                                cp_sb[:, NPAIR + m:NPAIR + m + 1])

            # ============ prologue: build u and z' =============
            def emit_premults():
                for m in range(MT):
                    xs = xcs_pool.tile([P, N], F32R, tag="xcs")
                    ring = nc.sync if m % 2 == 0 else nc.scalar
                    ring.dma_start(xs[:], xcm.ap()[:, m, :])
                    for cs4 in range(CHUNKS):
                        ns = slice(cs4 * NCHUNK, (cs4 + 1) * NCHUNK)
                        # identity term: u = c1 * x @ W0 + bias
                        u_ps = ps_tile("u_ps")
                        nc.tensor.matmul(u_ps[:], wx_sb[:, m, :], xs[:, ns],
                                         start=True, stop=True)
                        nc.scalar.activation(out=u_sb[:, m, ns], in_=u_ps[:],
                                             func=mybir.ActivationFunctionType.Identity,
                                             bias=bias_sb[:], scale=1.0)
                        # adjacency weights: z' = x @ (c2 W1g), then transpose.
                        # Evacuations are spread across Act and DVE so neither
                        # engine paces the premult pipeline (PE is ~0.9us/unit,
                        # a single engine doing two ~0.65us copies would stall
                        # it; the earlier all-Act version lost 1.3us/unit).
                        z_ps = ps_tile("z_ps")
                        nc.tensor.matmul(z_ps[:], wz_sb[:, m, :], xs[:, ns],
                                         start=True, stop=True)
                        zc_sb = zc_pool.tile([P, NCHUNK], F32, tag="zc")
                        nc.vector.tensor_copy(out=zc_sb[:], in_=z_ps[:])
                        tp_ps = ps_tile("tp_ps")
                        for q in range(NCHUNK // P):
                            nc.tensor.transpose(tp_ps[:, q * P:(q + 1) * P],
                                                zc_sb[:, q * P:(q + 1) * P], ident_sb[:])
                        tp4 = tp_ps.rearrange("p (q c) -> p q c", q=NCHUNK // P)
                        zn_out = znode_sb[:, 4 * cs4:4 * cs4 + 4, m * P:(m + 1) * P]
                        if cs4 % 2 == 0:
                            nc.scalar.copy(out=zn_out, in_=tp4[:, :, :])
                        else:
                            nc.vector.tensor_copy(out=zn_out, in_=tp4[:, :, :])

            # ============ DMA: adjacency streaming =============
            adjt_ap = adjt.ap().rearrange("(k p) n -> p k n", p=P)
            KB = 4   # k-tiles per batched DMA

            def load_at(ch, b):
                # split the 16.8MB/iter adjacency stream evenly across both
                # HWDGE rings (output writes are also split by bank parity),
                # ~142 GB/s per ring: a single ring saturates near ~205 GB/s
                # and measurably slowed the kernel.
                at_sb = adj_pool.tile([P, KB, NCHUNK], F32R, tag="at", name="at_sb")
                cs = slice(ch * NCHUNK, (ch + 1) * NCHUNK)
                ring = nc.sync if b < 2 else nc.scalar
                ring.dma_start(at_sb[:], adjt_ap[:, KB * b:KB * (b + 1), cs])
                return at_sb

            load_consts()
            emit_mlp()
            emit_premults()
            cur_at = [load_at(0, b) for b in range(KT // KB)]

            def znode_slice(k, m):
                return znode_sb[:, k, m * P:(m + 1) * P]

            # ============ main loop =============
            chunk_seq = [c for _ in range(repeat) for c in range(CHUNKS)]

            def emit_drain(ch, m, yt):
                cs = slice(ch * NCHUNK, (ch + 1) * NCHUNK)
                out_sb = outs_pool.tile([P, NCHUNK], F32, tag="outsb")
                nc.vector.tensor_add(out_sb[:], yt[:], u_sb[:, m, cs])
                ring = nc.sync if m % 2 == 0 else nc.scalar
                ring.dma_start(out_ap[:, m, cs], out_sb[:])

            for ci, ch in enumerate(chunk_seq):
                nxt = chunk_seq[ci + 1] if ci + 1 < len(chunk_seq) else None
                yt_ps = [ps_tile(f"yt{m}") for m in range(MT)]
                nxt_at = []
                for k in range(KT - 1):
                    if nxt is not None and k % KB == 0:
                        nxt_at.append(load_at(nxt, k // KB))
                    rhs = cur_at[k // KB][:, k % KB, :]
                    for m in range(MT):
                        nc.tensor.matmul(yt_ps[m][:], znode_slice(k, m), rhs,
                                         start=(k == 0), stop=False)
                # final k-round interleaved with DVE drains so each bank
                # frees as soon as its accumulation completes
                k = KT - 1
                rhs = cur_at[k // KB][:, k % KB, :]
                for m in range(MT):
                    nc.tensor.matmul(yt_ps[m][:], znode_slice(k, m), rhs,
                                     start=False, stop=True)
                    emit_drain(ch, m, yt_ps[m])
                cur_at = nxt_at if nxt is not None else None

    nc.finalize()
    return nc


_NC_CACHE = None


def _get_nc():
    global _NC_CACHE
    if _NC_CACHE is None:
        _NC_CACHE = build_kernel()
    return _NC_CACHE


def prep_in_maps(x, adj, st_emb, weights, bias, W1, b1, W2, b2):
    """Host-side layout prep -> per-core input dicts."""
    x = np.asarray(x, dtype=np.float32)
    adj = np.asarray(adj, dtype=np.float32)
    st_emb = np.asarray(st_emb, dtype=np.float32)
    weights = np.asarray(weights, dtype=np.float32)
    bias = np.asarray(bias, dtype=np.float32)
    W1 = np.asarray(W1, dtype=np.float32)
    b1 = np.asarray(b1, dtype=np.float32)
    W2 = np.asarray(W2, dtype=np.float32)
    b2 = np.asarray(b2, dtype=np.float32)

    adjT = np.ascontiguousarray(adj.T)
    w0g, w1g = weights[0], weights[1]                            # [64, 64] each
    z = np.zeros((64, 64), np.float32)
    wd0 = np.block([[w0g, z], [z, w0g]])                         # [128, 128]
    wd1 = np.block([[w1g, z], [z, w1g]])

    # pack the small constants into one [128, 688] blob (layout documented
    # at the cblob dram_tensor declaration in build_kernel)
    cblob = np.zeros((P, 688), np.float32)
    cblob[:64, 0:T] = st_emb.reshape(T, 64).T
    cblob[:64, 12:44] = W1
    cblob[:32, 44] = b1
    cblob[:32, 45] = W2[:, 0]
    cblob[0, 46] = b2[0]
    cblob[:, 47] = np.concatenate([bias, bias])
    cblob[0, 48:112] = 1.0                                       # mask_up
    cblob[0, 240:304] = 1.0                                      # mask_lo
    cblob[:, 304:432] = wd0
    cblob[:, 432:560] = wd1
    cblob[:, 560:688] = np.eye(P, dtype=np.float32)

    shared = {"adjt": adjT, "cblob": cblob}
    in_maps = []
    for b in range(B):
        xb = x[b]                                                # [T, N, C]
        # channel-major pair-slabs: xcm[a*64+c, m, n] = x[b, 2m+a, n, c]
        xcm = np.ascontiguousarray(
            xb.reshape(NPAIR, 2, N, 64).transpose(1, 3, 0, 2).reshape(P, NPAIR, N))
        in_maps.append({"xcm": xcm, **shared})
    return in_maps


def assemble_output(results):
    """Per-core [TC, N] f32 -> full [B, T, N, C] f32."""
    outs = []
    for r in results:
        oc = r["out"].reshape(T, 64, N).transpose(0, 2, 1)       # [T, N, 64]
        outs.append(oc)
    return np.stack(outs, axis=0).astype(np.float32)


def run(inputs, **spmd_kwargs):
    nc = _get_nc()
    in_maps = prep_in_maps(**inputs)
    res = run_bass_kernel_spmd(nc, in_maps, core_ids=list(range(B)), **spmd_kwargs)
    return assemble_output(res.results), res


def kernel(**inputs) -> np.ndarray:
    out, _ = run(inputs)
    return out
